# revision 3
# baseline (speedup 1.0000x reference)
"""MLDecoder classification head on 8 Trainium2 NeuronCores.

Sharding: data-parallel over batch B=64 -> 8 cores x 8 batches, all params
replicated. Inside each core a per-batch pipeline computes
  mem = relu(x @ We) ; cross-attention(q(query_embed), mem) ; FFN ; LN
and a final grouped-FC phase computes logits for the core's 8 batches.

Host-side prep is limited to layout transforms (transposes / reshapes /
sharding) and standard parameter folding (LN gains into adjacent weights,
softmax scale into wq, zero-sum bias absorption), all O(params).
"""

import numpy as np
import ml_dtypes

import concourse.bass as bass
import concourse.mybir as mybir
import concourse.tile as tile
from concourse import bacc
from concourse.masks import make_identity
from concourse.bass_utils import run_bass_kernel_spmd

# ---------------- problem dims (hardcoded) ----------------
B, C_IN, H, W = 64, 2048, 14, 14
D, FF, G, NCLS, NH = 768, 2048, 100, 9605, 8
DF = 97
HD = D // NH  # 96
S = H * W  # 196
EPS = 1e-5

N_CORES = 8
BL = B // N_CORES  # 8 batches per core

MODE = "bf16"  # "bf16" | "fp32r" | "fp32"

KC_C = C_IN // 128  # 16
KC_D = D // 128  # 6
KC_FF = FF // 128  # 16
NGRP = 5  # dup_pool groups per streamed chunk
NCHUNK_W = NGRP * DF  # 485

f32 = mybir.dt.float32
bf16 = mybir.dt.bfloat16


def _mode_cfg(mode):
    if mode == "bf16":
        return mybir.dt.bfloat16, ml_dtypes.bfloat16, S
    if mode == "fp32r":
        return mybir.dt.float32r, np.float32, 256
    if mode == "fp32":
        return mybir.dt.float32, np.float32, S
    raise ValueError(mode)


MM_DT, NP_DT, SP = _mode_cfg(MODE)
N_SBLK = (SP + 127) // 128
SBLK_ROWS = [min(128, SP - 128 * i) for i in range(N_SBLK)]


# ---------------- device kernel ----------------

def build_kernel():
    nc = bacc.Bacc("TRN2", target_bir_lowering=False)

    dm = MM_DT
    specs = [
        ("x", (BL, C_IN, SP), dm), ("wembT", (C_IN, D), dm),
        ("wkT", (D, D), dm), ("wvT", (D, D), dm), ("wqT", (D, D), dm),
        ("woT", (D, D), dm), ("w1T", (D, FF), dm), ("w2T", (FF, D), dm),
        ("dup", (D, G, DF), bf16), ("db2", (G, DF), bf16),
        ("qe", (G, D), f32), ("g1", (D,), f32), ("be1", (D,), f32),
        ("g2", (D,), f32), ("bemb", (D,), f32), ("bk", (D,), f32),
        ("bq", (D,), f32), ("bl1", (FF,), f32),
        ("bo_row", (1, D), dm), ("bl2_row", (1, D), dm),
        ("ones_mm", (1, 128), dm), ("ones_bf", (1, 128), bf16),
    ]
    hs = {n: nc.dram_tensor(n, shp, dt, kind="ExternalInput") for n, shp, dt in specs}
    hs["out"] = nc.dram_tensor("out", (BL, NCLS), f32, kind="ExternalOutput")

    with tile.TileContext(nc) as tc:
        _body(nc, tc, hs)
    nc.finalize()
    return nc


def _body(nc, tc, hs):
    from contextlib import ExitStack

    def dram(name):
        return hs[name][:]

    ctx = ExitStack()
    with ctx:
        const = ctx.enter_context(tc.tile_pool(name="const", bufs=1))
        dm = MM_DT

        # ---- resident weights ----
        wembT = const.tile([128, KC_C, D], dm)
        wv_emb = dram("wembT").rearrange("(kc p) d -> p kc d", p=128)
        for kc in range(KC_C):
            nc.sync.dma_start(out=wembT[:, kc, :], in_=wv_emb[:, kc, :])
        wkT = const.tile([128, KC_D, D], dm)
        nc.sync.dma_start(out=wkT, in_=dram("wkT").rearrange("(kc p) e -> p kc e", p=128))
        wvT = const.tile([128, KC_D, D], dm)
        nc.sync.dma_start(out=wvT, in_=dram("wvT").rearrange("(kc p) e -> p kc e", p=128))
        wqT = const.tile([128, KC_D, D], dm)
        nc.sync.dma_start(out=wqT, in_=dram("wqT").rearrange("(kc p) e -> p kc e", p=128))
        # out-proj consumes k (=d) in 96-row blocks matching ctxT head tiles
        woT = const.tile([96, NH, D], dm)
        nc.sync.dma_start(out=woT, in_=dram("woT").rearrange("(h p) e -> p h e", p=96))
        w1T = const.tile([128, KC_D, FF], dm)
        nc.sync.dma_start(out=w1T, in_=dram("w1T").rearrange("(kc p) f -> p kc f", p=128))
        w2T = const.tile([128, KC_FF, D], dm)
        nc.sync.dma_start(out=w2T, in_=dram("w2T").rearrange("(kc p) e -> p kc e", p=128))

        # ---- small constants ----
        bemb = const.tile([128, KC_D], f32)
        nc.sync.dma_start(out=bemb, in_=dram("bemb").rearrange("(c p) -> p c", p=128))
        bk = const.tile([96, NH], f32)
        nc.sync.dma_start(out=bk, in_=dram("bk").rearrange("(h p) -> p h", p=96))
        bq = const.tile([96, NH], f32)
        nc.sync.dma_start(out=bq, in_=dram("bq").rearrange("(h p) -> p h", p=96))
        bl1 = const.tile([128, KC_FF], f32)
        nc.sync.dma_start(out=bl1, in_=dram("bl1").rearrange("(c p) -> p c", p=128))
        bo_row = const.tile([1, D], dm)
        nc.sync.dma_start(out=bo_row, in_=dram("bo_row"))
        bl2_row = const.tile([1, D], dm)
        nc.sync.dma_start(out=bl2_row, in_=dram("bl2_row"))
        ones_mm = const.tile([1, 128], dm)
        nc.sync.dma_start(out=ones_mm, in_=dram("ones_mm"))
        ones_bf = const.tile([1, 128], bf16)
        nc.sync.dma_start(out=ones_bf, in_=dram("ones_bf"))

        def bcast(ap, n_part):
            return bass.AP(tensor=ap.tensor, offset=ap.offset,
                           ap=[[0, n_part]] + list(ap.ap))

        g2b = const.tile([G, D], f32)
        nc.sync.dma_start(out=g2b, in_=bcast(dram("g2"), G))

        ident = const.tile([128, 128], f32)
        make_identity(nc, ident)
        eps_t = const.tile([128, 1], f32)
        nc.vector.memset(eps_t, EPS)

        smal = ctx.enter_context(tc.tile_pool(name="smal", bufs=4))
        ps = ctx.enter_context(tc.tile_pool(name="ps", bufs=8, space="PSUM"))

        # ---------- helpers ----------
        def layernorm_core(out_sb, in_sb, tag):
            """out = (in - mean)/sqrt(var+EPS), rows [G, D]."""
            st = smal.tile([G, 3, 6], f32, tag=tag + "_st")
            iv = in_sb.rearrange("g (n f) -> g n f", f=256)
            for i in range(3):
                nc.vector.bn_stats(out=st[:, i, :], in_=iv[:, i, :])
            mv = smal.tile([G, 2], f32, tag=tag + "_mv")
            nc.vector.bn_aggr(out=mv, in_=st)
            sd = smal.tile([G, 1], f32, tag=tag + "_sd")
            nc.scalar.activation(out=sd, in_=mv[:, 1:2],
                                 func=mybir.ActivationFunctionType.Sqrt,
                                 bias=eps_t[:G], scale=1.0)
            nc.vector.reciprocal(out=sd, in_=sd)
            nc.vector.tensor_scalar(out=out_sb, in0=in_sb,
                                    scalar1=mv[:, 0:1], scalar2=sd,
                                    op0=mybir.AluOpType.subtract,
                                    op1=mybir.AluOpType.mult)

        def transpose_cols(dst_ap, src_sb, kc, rows=128):
            """dst (=[rows, P]) = src[:, kc*128 : kc*128+rows].T ; dst dtype rounds."""
            p_t = ps.tile([128, 512], f32, tag="ps")
            pn = src_sb.shape[0]
            nc.tensor.transpose(p_t[:rows, :pn], src_sb[:, kc * 128:kc * 128 + rows],
                                ident[:pn, :pn])
            nc.scalar.activation(out=dst_ap, in_=p_t[:rows, :pn],
                                 func=mybir.ActivationFunctionType.Copy,
                                 bias=0.0, scale=1.0)

        # ---------- phase 0: tgt_n and qT (batch independent) ----------
        tgt_n = const.tile([G, D], f32)
        tnT = const.tile([128, KC_D, G], dm)
        qT = const.tile([96, NH, G], dm)
        with tc.tile_pool(name="tmp0", bufs=1) as tmp0:
            qe_sb = tmp0.tile([G, D], f32)
            nc.sync.dma_start(out=qe_sb, in_=dram("qe"))
            g1b = tmp0.tile([G, D], f32)
            nc.sync.dma_start(out=g1b, in_=bcast(dram("g1"), G))
            be1b = tmp0.tile([G, D], f32)
            nc.sync.dma_start(out=be1b, in_=bcast(dram("be1"), G))
            qe2 = tmp0.tile([G, D], f32)
            nc.scalar.mul(qe2, qe_sb, 2.0)
            tnc = tmp0.tile([G, D], f32)
            layernorm_core(tnc, qe2, "ln1")
            nc.vector.tensor_mul(tgt_n, tnc, g1b)
            nc.vector.tensor_add(tgt_n, tgt_n, be1b)

            for kc in range(KC_D):
                transpose_cols(tnT[:, kc, :], tgt_n, kc)

            for h in range(NH):
                p_q = ps.tile([128, 512], f32, tag="ps")
                for kc in range(KC_D):
                    nc.tensor.matmul(p_q[:96, :G], wqT[:, kc, h * 96:(h + 1) * 96],
                                     tnT[:, kc, :], start=(kc == 0), stop=(kc == KC_D - 1))
                nc.vector.tensor_scalar(out=qT[:, h, :], in0=p_q[:96, :G],
                                        scalar1=bq[:, h:h + 1], scalar2=None,
                                        op0=mybir.AluOpType.add)

        work = ctx.enter_context(tc.tile_pool(name="work", bufs=2))
        xpool = ctx.enter_context(tc.tile_pool(name="xpool", bufs=2 * KC_C))
        lnpool = ctx.enter_context(tc.tile_pool(name="lnpool", bufs=3))
        hpool = ctx.enter_context(tc.tile_pool(name="hpool", bufs=1))
        dupp = ctx.enter_context(tc.tile_pool(name="dupp", bufs=2))

        # ---------- per-batch pipeline ----------
        hT = hpool.tile([128, KC_D, BL, G], bf16)
        x_view = dram("x").rearrange("b (kc p) s -> b p kc s", p=128)

        for b in range(BL):
            xs = []
            for kc in range(KC_C):
                x_kc = xpool.tile([128, SP], dm, tag="x")
                nc.sync.dma_start(out=x_kc, in_=x_view[b][:, kc, :])
                xs.append(x_kc)

            # embed: memT[d, s] = relu(sum_c wembT[c, d] * x[c, s] + bemb)
            memT = work.tile([128, KC_D, SP], dm, tag="memT")
            for dblk in range(KC_D):
                p_e = ps.tile([128, 512], f32, tag="ps")
                for kc in range(KC_C):
                    nc.tensor.matmul(p_e[:, :SP], wembT[:, kc, dblk * 128:(dblk + 1) * 128],
                                     xs[kc], start=(kc == 0), stop=(kc == KC_C - 1))
                nc.scalar.activation(out=memT[:, dblk, :], in_=p_e[:, :SP],
                                     func=mybir.ActivationFunctionType.Relu,
                                     bias=bemb[:, dblk:dblk + 1], scale=1.0)

            # kT[e, s] per head block
            kT = work.tile([96, NH, SP], dm, tag="kT")
            for h in range(NH):
                p_k = ps.tile([128, 512], f32, tag="ps")
                for kc in range(KC_D):
                    nc.tensor.matmul(p_k[:96, :SP], wkT[:, kc, h * 96:(h + 1) * 96],
                                     memT[:, kc, :], start=(kc == 0), stop=(kc == KC_D - 1))
                nc.vector.tensor_scalar(out=kT[:, h, :], in0=p_k[:96, :SP],
                                        scalar1=bk[:, h:h + 1], scalar2=None,
                                        op0=mybir.AluOpType.add)

            # v[s, e] natural layout (bias bv absorbed into bo on host)
            v_sb = work.tile([128, N_SBLK, D], dm, tag="v")
            for sblk in range(N_SBLK):
                rows = SBLK_ROWS[sblk]
                for nch in range(2):
                    p_v = ps.tile([128, 512], f32, tag="ps")
                    for kc in range(KC_D):
                        nc.tensor.matmul(
                            p_v[:rows, :384],
                            memT[:, kc, sblk * 128:sblk * 128 + rows],
                            wvT[:, kc, nch * 384:(nch + 1) * 384],
                            start=(kc == 0), stop=(kc == KC_D - 1))
                    nc.vector.tensor_copy(
                        out=v_sb[:rows, sblk, nch * 384:(nch + 1) * 384],
                        in_=p_v[:rows, :384])

            # attention per head
            ctxT = work.tile([96, NH, G], dm, tag="ctxT")
            for h in range(NH):
                p_s = ps.tile([128, 512], f32, tag="ps")
                nc.tensor.matmul(p_s[:G, :SP], qT[:, h, :], kT[:, h, :],
                                 start=True, stop=True)
                if SP > S:
                    nc.vector.memset(p_s[:G, S:SP], -1e30)
                negmax = smal.tile([G, 1], f32, tag="negmax")
                nc.vector.reduce_max(negmax, p_s[:G, :SP],
                                     axis=mybir.AxisListType.X, negate=True)
                p_sb = work.tile([G, SP], f32, tag="p_sb")
                rsum = smal.tile([G, 1], f32, tag="rsum")
                nc.scalar.activation(out=p_sb, in_=p_s[:G, :SP],
                                     func=mybir.ActivationFunctionType.Exp,
                                     bias=negmax, scale=1.0, accum_out=rsum)
                nc.vector.reciprocal(out=rsum, in_=rsum)
                nc.vector.tensor_scalar_mul(p_sb, p_sb, rsum)
                attnT = work.tile([128, N_SBLK, G], dm, tag="attnT")
                for sblk in range(N_SBLK):
                    transpose_cols(attnT[:SBLK_ROWS[sblk], sblk, :], p_sb, sblk,
                                   rows=SBLK_ROWS[sblk])
                p_c = ps.tile([128, 512], f32, tag="ps")
                for sblk in range(N_SBLK):
                    rows = SBLK_ROWS[sblk]
                    nc.tensor.matmul(p_c[:96, :G],
                                     v_sb[:rows, sblk, h * 96:(h + 1) * 96],
                                     attnT[:rows, sblk, :],
                                     start=(sblk == 0), stop=(sblk == N_SBLK - 1))
                nc.vector.tensor_copy(out=ctxT[:, h, :], in_=p_c[:96, :G])

            # out-proj + residual -> t2 ; LN2 core
            t2 = lnpool.tile([G, D], f32, tag="lnbuf")
            for nch in range(2):
                p_o = ps.tile([128, 512], f32, tag="ps")
                nc.tensor.matmul(p_o[:G, :384], ones_mm[:, :G],
                                 bo_row[:, nch * 384:(nch + 1) * 384],
                                 start=True, stop=False)
                for h in range(NH):
                    nc.tensor.matmul(p_o[:G, :384], ctxT[:, h, :],
                                     woT[:, h, nch * 384:(nch + 1) * 384],
                                     start=False, stop=(h == NH - 1))
                nc.vector.tensor_add(t2[:, nch * 384:(nch + 1) * 384],
                                     tgt_n[:, nch * 384:(nch + 1) * 384],
                                     p_o[:G, :384])
            lnc2 = lnpool.tile([G, D], f32, tag="lnbuf")
            layernorm_core(lnc2, t2, "ln2")

            # FFN1 (g2 folded into w1T on host): ffT[f, g]
            lnc2T = work.tile([128, KC_D, G], dm, tag="lnc2T")
            for kc in range(KC_D):
                transpose_cols(lnc2T[:, kc, :], lnc2, kc)
            ffT = work.tile([128, KC_FF, G], dm, tag="ffT")
            for fblk in range(KC_FF):
                p_f = ps.tile([128, 512], f32, tag="ps")
                for kc in range(KC_D):
                    nc.tensor.matmul(p_f[:, :G], w1T[:, kc, fblk * 128:(fblk + 1) * 128],
                                     lnc2T[:, kc, :], start=(kc == 0), stop=(kc == KC_D - 1))
                nc.scalar.activation(out=ffT[:, fblk, :], in_=p_f[:, :G],
                                     func=mybir.ActivationFunctionType.Relu,
                                     bias=bl1[:, fblk:fblk + 1], scale=1.0)

            # FFN2 + h_pre = lnc2*g2 + (ff + bl2 + be2)
            for nch in range(2):
                p_2 = ps.tile([128, 512], f32, tag="ps")
                nc.tensor.matmul(p_2[:G, :384], ones_mm[:, :G],
                                 bl2_row[:, nch * 384:(nch + 1) * 384],
                                 start=True, stop=False)
                for kc in range(KC_FF):
                    nc.tensor.matmul(p_2[:G, :384], ffT[:, kc, :],
                                     w2T[:, kc, nch * 384:(nch + 1) * 384],
                                     start=False, stop=(kc == KC_FF - 1))
                sl = slice(nch * 384, (nch + 1) * 384)
                nc.vector.tensor_mul(t2[:, sl], lnc2[:, sl], g2b[:, sl])
                nc.vector.tensor_add(t2[:, sl], t2[:, sl], p_2[:G, :384])
            lnc3 = lnpool.tile([G, D], f32, tag="lnbuf")
            layernorm_core(lnc3, t2, "ln3")
            for kc in range(KC_D):
                transpose_cols(hT[:, kc, b, :], lnc3, kc)

        # ---------- grouped FC ----------
        dup_view = dram("dup").rearrange("(kc p) g f -> p kc g f", p=128)
        out_flat = dram("out")
        for gc in range(G // NGRP):
            dup_sb = dupp.tile([128, KC_D, NGRP, DF], bf16, tag="dup")
            nc.sync.dma_start(out=dup_sb,
                              in_=dup_view[:, :, gc * NGRP:(gc + 1) * NGRP, :])
            db2_sb = dupp.tile([1, NGRP, DF], bf16, tag="db2c")
            nc.sync.dma_start(out=db2_sb, in_=dram("db2")[gc * NGRP:(gc + 1) * NGRP, :])
            lchunk = work.tile([BL, NGRP, DF], f32, tag="lchunk")
            for gi in range(NGRP):
                g = gc * NGRP + gi
                p_g = ps.tile([128, 512], f32, tag="ps")
                nc.tensor.matmul(p_g[:BL, :DF], ones_bf[:, :BL],
                                 db2_sb[:, gi, :], start=True, stop=False)
                for kc in range(KC_D):
                    nc.tensor.matmul(p_g[:BL, :DF], hT[:, kc, :, g],
                                     dup_sb[:, kc, gi, :],
                                     start=False, stop=(kc == KC_D - 1))
                nc.vector.tensor_copy(out=lchunk[:, gi, :], in_=p_g[:BL, :DF])
            c0 = gc * NCHUNK_W
            wout = min(NCHUNK_W, NCLS - c0)
            nc.sync.dma_start(
                out=out_flat[:, c0:c0 + wout],
                in_=lchunk.rearrange("b gi f -> b (gi f)")[:, :wout])


# ---------------- host side ----------------

_CACHED = {}


def _prep_inputs(inputs):
    f = np.float64
    w_embed = inputs["w_embed"].astype(f)
    wq, wk, wv, wo = (inputs[k].astype(f) for k in ("wq", "wk", "wv", "wo"))
    bq, bk, bv, bo = (inputs[k].astype(f) for k in ("bq", "bk", "bv", "bo"))
    g1, be1 = inputs["g1"].astype(f), inputs["be1"].astype(f)
    g2, be2 = inputs["g2"].astype(f), inputs["be2"].astype(f)
    g3, be3 = inputs["g3"].astype(f), inputs["be3"].astype(f)
    w1, bl1 = inputs["w1"].astype(f), inputs["bl1"].astype(f)
    w2, bl2 = inputs["w2"].astype(f), inputs["bl2"].astype(f)
    dup_pool = inputs["dup_pool"].astype(f)
    dup_bias = inputs["dup_bias"].astype(f)

    sc = 1.0 / np.sqrt(HD)
    bo_eff = bo + wo @ bv
    bl1_eff = bl1 + w1 @ be2
    bl2_eff = bl2 + be2
    dup2 = dup_pool.transpose(1, 0, 2) * g3[:, None, None]  # [D, G, DF]
    db2 = np.concatenate([dup_bias, np.zeros(G * DF - NCLS)])  # [G*DF]
    db2 = db2 + np.einsum("d,gdf->gf", be3, dup_pool).reshape(-1)
    w1T_eff = (w1 * g2[None, :]).T  # [D, FF]

    x = np.ascontiguousarray(inputs["x"].reshape(B, C_IN, S))
    if SP > S:
        x = np.concatenate([x, np.zeros((B, C_IN, SP - S), np.float32)], axis=2)

    np32 = np.float32
    base = {
        "wembT": np.ascontiguousarray(w_embed.T).astype(NP_DT),
        "wkT": np.ascontiguousarray(wk.T).astype(NP_DT),
        "wvT": np.ascontiguousarray(wv.T).astype(NP_DT),
        "wqT": np.ascontiguousarray(wq.T * sc).astype(NP_DT),
        "woT": np.ascontiguousarray(wo.T).astype(NP_DT),
        "w1T": np.ascontiguousarray(w1T_eff).astype(NP_DT),
        "w2T": np.ascontiguousarray(w2.T).astype(NP_DT),
        "dup": np.ascontiguousarray(dup2).astype(ml_dtypes.bfloat16),
        "db2": db2.reshape(G, DF).astype(ml_dtypes.bfloat16),
        "qe": inputs["query_embed"].astype(np32),
        "g1": g1.astype(np32), "be1": be1.astype(np32), "g2": g2.astype(np32),
        "bemb": inputs["b_embed"].astype(np32),
        "bk": bk.astype(np32),
        "bq": (bq * sc).astype(np32),
        "bl1": bl1_eff.astype(np32),
        "bo_row": bo_eff.astype(NP_DT).reshape(1, D),
        "bl2_row": bl2_eff.astype(NP_DT).reshape(1, D),
        "ones_mm": np.ones((1, 128), NP_DT),
        "ones_bf": np.ones((1, 128), ml_dtypes.bfloat16),
    }
    in_maps = []
    for c in range(N_CORES):
        m = dict(base)
        m["x"] = np.ascontiguousarray(x[c * BL:(c + 1) * BL]).astype(NP_DT)
        in_maps.append(m)
    return in_maps


def get_nc():
    if "nc" not in _CACHED:
        _CACHED["nc"] = build_kernel()
    return _CACHED["nc"]


def kernel(**inputs) -> np.ndarray:
    nc = get_nc()
    in_maps = _prep_inputs(inputs)
    res = run_bass_kernel_spmd(nc, in_maps, core_ids=list(range(N_CORES)))
    return np.concatenate([res.results[c]["out"] for c in range(N_CORES)], axis=0)


# revision 6
# speedup vs baseline: 1.1204x; 1.1204x over previous
"""MLDecoder classification head on 8 Trainium2 NeuronCores.

Sharding: data-parallel over batch B=64 -> 8 cores x 8 batches, all params
replicated. Inside each core a per-batch pipeline computes
  mem = relu(x @ We) ; cross-attention(q(query_embed), mem) ; FFN ; LN
and a final grouped-FC phase computes logits for the core's 8 batches.

Host-side prep is limited to layout transforms (transposes / reshapes /
sharding) and standard parameter folding (LN gains into adjacent weights,
softmax scale into wq, zero-sum bias absorption), all O(params).
"""

import numpy as np
import ml_dtypes

import concourse.bass as bass
import concourse.mybir as mybir
import concourse.tile as tile
from concourse import bacc
from concourse.masks import make_identity
from concourse.bass_utils import run_bass_kernel_spmd

# ---------------- problem dims (hardcoded) ----------------
B, C_IN, H, W = 64, 2048, 14, 14
D, FF, G, NCLS, NH = 768, 2048, 100, 9605, 8
DF = 97
HD = D // NH  # 96
S = H * W  # 196
EPS = 1e-5

N_CORES = 8
BL = B // N_CORES  # 8 batches per core

MODE = "bf16"  # "bf16" | "fp32r" | "fp32"

KC_C = C_IN // 128  # 16
KC_D = D // 128  # 6
KC_FF = FF // 128  # 16
NGRP = 8  # dup_pool groups per streamed chunk
NCHUNK_W = NGRP * DF  # 485

f32 = mybir.dt.float32
bf16 = mybir.dt.bfloat16


def _mode_cfg(mode):
    if mode == "bf16":
        return mybir.dt.bfloat16, ml_dtypes.bfloat16, S
    if mode == "fp32r":
        return mybir.dt.float32r, np.float32, 256
    if mode == "fp32":
        return mybir.dt.float32, np.float32, S
    raise ValueError(mode)


MM_DT, NP_DT, SP = _mode_cfg(MODE)
N_SBLK = (SP + 127) // 128
SBLK_ROWS = [min(128, SP - 128 * i) for i in range(N_SBLK)]


# ---------------- device kernel ----------------

def build_kernel():
    nc = bacc.Bacc("TRN2", target_bir_lowering=False)

    dm = MM_DT
    specs = [
        ("x", (BL, C_IN, SP), dm), ("wembT", (C_IN, D), dm),
        ("wkT", (D, D), dm), ("wvT", (D, D), dm), ("wqT", (D, D), dm),
        ("woT", (D, D), dm), ("w1T", (D, FF), dm), ("w2T", (FF, D), dm),
        ("dup", (D, G, DF), bf16), ("db2", (G, DF), bf16),
        ("qe", (G, D), f32), ("g1", (D,), f32), ("be1", (D,), f32),
        ("g2", (D,), f32), ("bemb", (D,), f32), ("bk", (D,), f32),
        ("bq", (D,), f32), ("bl1", (FF,), f32),
        ("bo_row", (1, D), dm), ("bl2_row", (1, D), dm),
        ("ones_mm", (1, 128), dm), ("ones_bf", (1, 128), bf16),
    ]
    hs = {n: nc.dram_tensor(n, shp, dt, kind="ExternalInput") for n, shp, dt in specs}
    hs["out"] = nc.dram_tensor("out", (BL, NCLS), f32, kind="ExternalOutput")

    with tile.TileContext(nc) as tc:
        _body(nc, tc, hs)
    nc.finalize()
    return nc


def _body(nc, tc, hs):
    from contextlib import ExitStack

    def dram(name):
        return hs[name][:]

    ctx = ExitStack()
    with ctx:
        const = ctx.enter_context(tc.tile_pool(name="const", bufs=1))
        dm = MM_DT

        # ---- small constants first (phase0 critical path), then big weights ----
        bemb = const.tile([128, KC_D], f32)
        nc.sync.dma_start(out=bemb, in_=dram("bemb").rearrange("(c p) -> p c", p=128))
        bk = const.tile([96, NH], f32)
        nc.sync.dma_start(out=bk, in_=dram("bk").rearrange("(h p) -> p h", p=96))
        bq = const.tile([96, NH], f32)
        nc.sync.dma_start(out=bq, in_=dram("bq").rearrange("(h p) -> p h", p=96))
        bl1 = const.tile([128, KC_FF], f32)
        nc.sync.dma_start(out=bl1, in_=dram("bl1").rearrange("(c p) -> p c", p=128))
        bo_row = const.tile([1, D], dm)
        nc.sync.dma_start(out=bo_row, in_=dram("bo_row"))
        bl2_row = const.tile([1, D], dm)
        nc.sync.dma_start(out=bl2_row, in_=dram("bl2_row"))
        ones_mm = const.tile([1, 128], dm)
        nc.sync.dma_start(out=ones_mm, in_=dram("ones_mm"))
        ones_bf = const.tile([1, 128], bf16)
        nc.sync.dma_start(out=ones_bf, in_=dram("ones_bf"))

        def bcast(ap, n_part):
            return bass.AP(tensor=ap.tensor, offset=ap.offset,
                           ap=[[0, n_part]] + list(ap.ap))

        g2b = const.tile([G, D], f32)
        nc.sync.dma_start(out=g2b, in_=bcast(dram("g2"), G))

        # big weights: embed first, then attention, then FFN weights on the
        # scalar queue (needed latest). wqT lives in the scoped phase-0 pool.
        wembT = const.tile([128, KC_C, D], dm)
        wv_emb = dram("wembT").rearrange("(kc p) d -> p kc d", p=128)
        for kc in range(KC_C):
            nc.sync.dma_start(out=wembT[:, kc, :], in_=wv_emb[:, kc, :])
        wkT = const.tile([128, KC_D, D], dm)
        nc.sync.dma_start(out=wkT, in_=dram("wkT").rearrange("(kc p) e -> p kc e", p=128))
        wvT = const.tile([128, KC_D, D], dm)
        nc.sync.dma_start(out=wvT, in_=dram("wvT").rearrange("(kc p) e -> p kc e", p=128))
        # out-proj consumes k (=d) in 96-row blocks matching ctxT head tiles
        woT = const.tile([96, NH, D], dm)
        nc.sync.dma_start(out=woT, in_=dram("woT").rearrange("(h p) e -> p h e", p=96))
        w1T = const.tile([128, KC_D, FF], dm)
        nc.scalar.dma_start(out=w1T, in_=dram("w1T").rearrange("(kc p) f -> p kc f", p=128))
        w2T = const.tile([128, KC_FF, D], dm)
        nc.scalar.dma_start(out=w2T, in_=dram("w2T").rearrange("(kc p) e -> p kc e", p=128))

        ident = const.tile([128, 128], f32)
        make_identity(nc, ident)
        eps_t = const.tile([128, 1], f32)
        nc.vector.memset(eps_t, EPS)

        smal = ctx.enter_context(tc.tile_pool(name="smal", bufs=4))
        ps = ctx.enter_context(tc.tile_pool(name="ps", bufs=8, space="PSUM"))

        # ---------- helpers ----------
        def layernorm_core(out_sb, in_sb, tag):
            """out = (in - mean)/sqrt(var+EPS), rows [G, D]."""
            st = smal.tile([G, 3, 6], f32, tag=tag + "_st")
            iv = in_sb.rearrange("g (n f) -> g n f", f=256)
            for i in range(3):
                nc.vector.bn_stats(out=st[:, i, :], in_=iv[:, i, :])
            mv = smal.tile([G, 2], f32, tag=tag + "_mv")
            nc.vector.bn_aggr(out=mv, in_=st)
            sd = smal.tile([G, 1], f32, tag=tag + "_sd")
            nc.scalar.activation(out=sd, in_=mv[:, 1:2],
                                 func=mybir.ActivationFunctionType.Sqrt,
                                 bias=eps_t[:G], scale=1.0)
            nc.vector.reciprocal(out=sd, in_=sd)
            nc.vector.tensor_scalar(out=out_sb, in0=in_sb,
                                    scalar1=mv[:, 0:1], scalar2=sd,
                                    op0=mybir.AluOpType.subtract,
                                    op1=mybir.AluOpType.mult)

        def transpose_cols(dst_ap, src_sb, kc, rows=128):
            """dst (=[rows, P]) = src[:, kc*128 : kc*128+rows].T ; dst dtype rounds."""
            p_t = ps.tile([128, 512], f32, tag="ps")
            pn = src_sb.shape[0]
            nc.tensor.transpose(p_t[:rows, :pn], src_sb[:, kc * 128:kc * 128 + rows],
                                ident[:pn, :pn])
            nc.scalar.activation(out=dst_ap, in_=p_t[:rows, :pn],
                                 func=mybir.ActivationFunctionType.Copy,
                                 bias=0.0, scale=1.0)

        # ---------- phase 0: tgt_n and qT (batch independent) ----------
        tgt_n = const.tile([G, D], f32)
        tnT = const.tile([128, KC_D, G], dm)
        qT = const.tile([96, NH, G], dm)
        with tc.tile_pool(name="tmp0", bufs=1) as tmp0:
            wqT = tmp0.tile([128, KC_D, D], dm)
            nc.sync.dma_start(out=wqT, in_=dram("wqT").rearrange("(kc p) e -> p kc e", p=128))
            qe_sb = tmp0.tile([G, D], f32)
            nc.sync.dma_start(out=qe_sb, in_=dram("qe"))
            g1b = tmp0.tile([G, D], f32)
            nc.sync.dma_start(out=g1b, in_=bcast(dram("g1"), G))
            be1b = tmp0.tile([G, D], f32)
            nc.sync.dma_start(out=be1b, in_=bcast(dram("be1"), G))
            qe2 = tmp0.tile([G, D], f32)
            nc.scalar.mul(qe2, qe_sb, 2.0)
            tnc = tmp0.tile([G, D], f32)
            layernorm_core(tnc, qe2, "ln1")
            nc.vector.tensor_mul(tgt_n, tnc, g1b)
            nc.vector.tensor_add(tgt_n, tgt_n, be1b)

            for kc in range(KC_D):
                transpose_cols(tnT[:, kc, :], tgt_n, kc)

            for h in range(NH):
                p_q = ps.tile([128, 512], f32, tag="ps")
                for kc in range(KC_D):
                    nc.tensor.matmul(p_q[:96, :G], wqT[:, kc, h * 96:(h + 1) * 96],
                                     tnT[:, kc, :], start=(kc == 0), stop=(kc == KC_D - 1))
                nc.vector.tensor_scalar(out=qT[:, h, :], in0=p_q[:96, :G],
                                        scalar1=bq[:, h:h + 1], scalar2=None,
                                        op0=mybir.AluOpType.add)

        work = ctx.enter_context(tc.tile_pool(name="work", bufs=2))
        xpool = ctx.enter_context(tc.tile_pool(name="xpool", bufs=20))
        lnpool = ctx.enter_context(tc.tile_pool(name="lnpool", bufs=3))
        hpool = ctx.enter_context(tc.tile_pool(name="hpool", bufs=1))
        dupp = ctx.enter_context(tc.tile_pool(name="dupp", bufs=2))

        # ---------- per-batch pipeline (two-stage software pipeline) ----------
        hT = hpool.tile([128, KC_D, BL, G], bf16)
        x_view = dram("x").rearrange("b (kc p) s -> b p kc s", p=128)
        state = {}

        def emit_x(b):
            xs = []
            for kc in range(KC_C):
                x_kc = xpool.tile([128, SP], dm, tag="x")
                nc.gpsimd.dma_start(out=x_kc, in_=x_view[b][:, kc, :])
                xs.append(x_kc)
            return xs

        def emit_embed(b, xs):
            # embed: memT[d, s] = relu(sum_c wembT[c, d] * x[c, s] + bemb)
            memT = work.tile([128, KC_D, SP], dm, tag="memT")
            for dblk in range(KC_D):
                p_e = ps.tile([128, 512], f32, tag="ps")
                for kc in range(KC_C):
                    nc.tensor.matmul(p_e[:, :SP], wembT[:, kc, dblk * 128:(dblk + 1) * 128],
                                     xs[kc], start=(kc == 0), stop=(kc == KC_C - 1))
                nc.scalar.activation(out=memT[:, dblk, :], in_=p_e[:, :SP],
                                     func=mybir.ActivationFunctionType.Relu,
                                     bias=bemb[:, dblk:dblk + 1], scale=1.0)
            return memT

        def emit_kT(b, memT):
            kT = work.tile([96, NH, SP], dm, tag="kT")
            for h in range(NH):
                p_k = ps.tile([128, 512], f32, tag="ps")
                for kc in range(KC_D):
                    nc.tensor.matmul(p_k[:96, :SP], wkT[:, kc, h * 96:(h + 1) * 96],
                                     memT[:, kc, :], start=(kc == 0), stop=(kc == KC_D - 1))
                nc.vector.tensor_scalar(out=kT[:, h, :], in0=p_k[:96, :SP],
                                        scalar1=bk[:, h:h + 1], scalar2=None,
                                        op0=mybir.AluOpType.add)
            return kT

        def emit_v(b, memT):
            v_sb = work.tile([128, N_SBLK, D], dm, tag="v")
            for sblk in range(N_SBLK):
                rows = SBLK_ROWS[sblk]
                for nch in range(2):
                    p_v = ps.tile([128, 512], f32, tag="ps")
                    for kc in range(KC_D):
                        nc.tensor.matmul(
                            p_v[:rows, :384],
                            memT[:, kc, sblk * 128:sblk * 128 + rows],
                            wvT[:, kc, nch * 384:(nch + 1) * 384],
                            start=(kc == 0), stop=(kc == KC_D - 1))
                    nc.vector.tensor_copy(
                        out=v_sb[:rows, sblk, nch * 384:(nch + 1) * 384],
                        in_=p_v[:rows, :384])
            return v_sb

        def emit_attn(b, kT, v_sb):
            ctxT = work.tile([96, NH, G], dm, tag="ctxT")
            for h in range(NH):
                p_s = ps.tile([128, 512], f32, tag="ps")
                nc.tensor.matmul(p_s[:G, :SP], qT[:, h, :], kT[:, h, :],
                                 start=True, stop=True)
                if SP > S:
                    nc.vector.memset(p_s[:G, S:SP], -1e30)
                negmax = smal.tile([G, 1], f32, tag="negmax")
                nc.vector.reduce_max(negmax, p_s[:G, :SP],
                                     axis=mybir.AxisListType.X, negate=True)
                p_sb = work.tile([G, SP], f32, tag="p_sb")
                rsum = smal.tile([G, 1], f32, tag="rsum")
                nc.scalar.activation(out=p_sb, in_=p_s[:G, :SP],
                                     func=mybir.ActivationFunctionType.Exp,
                                     bias=negmax, scale=1.0, accum_out=rsum)
                nc.vector.reciprocal(out=rsum, in_=rsum)
                nc.vector.tensor_scalar_mul(p_sb, p_sb, rsum)
                attnT = work.tile([128, N_SBLK, G], dm, tag="attnT")
                for sblk in range(N_SBLK):
                    transpose_cols(attnT[:SBLK_ROWS[sblk], sblk, :], p_sb, sblk,
                                   rows=SBLK_ROWS[sblk])
                p_c = ps.tile([128, 512], f32, tag="ps")
                for sblk in range(N_SBLK):
                    rows = SBLK_ROWS[sblk]
                    nc.tensor.matmul(p_c[:96, :G],
                                     v_sb[:rows, sblk, h * 96:(h + 1) * 96],
                                     attnT[:rows, sblk, :],
                                     start=(sblk == 0), stop=(sblk == N_SBLK - 1))
                nc.vector.tensor_copy(out=ctxT[:, h, :], in_=p_c[:96, :G])
            return ctxT

        def emit_oproj_ln2(b, ctxT):
            t2 = lnpool.tile([G, D], f32, tag="lnbuf")
            for nch in range(2):
                p_o = ps.tile([128, 512], f32, tag="ps")
                nc.tensor.matmul(p_o[:G, :384], ones_mm[:, :G],
                                 bo_row[:, nch * 384:(nch + 1) * 384],
                                 start=True, stop=False)
                for h in range(NH):
                    nc.tensor.matmul(p_o[:G, :384], ctxT[:, h, :],
                                     woT[:, h, nch * 384:(nch + 1) * 384],
                                     start=False, stop=(h == NH - 1))
                nc.vector.tensor_add(t2[:, nch * 384:(nch + 1) * 384],
                                     tgt_n[:, nch * 384:(nch + 1) * 384],
                                     p_o[:G, :384])
            lnc2 = lnpool.tile([G, D], f32, tag="lnbuf")
            layernorm_core(lnc2, t2, "ln2")
            lnc2T = work.tile([128, KC_D, G], dm, tag="lnc2T")
            for kc in range(KC_D):
                transpose_cols(lnc2T[:, kc, :], lnc2, kc)
            return t2, lnc2, lnc2T

        def emit_ffn1(b, lnc2T):
            ffT = work.tile([128, KC_FF, G], dm, tag="ffT")
            for fblk in range(KC_FF):
                p_f = ps.tile([128, 512], f32, tag="ps")
                for kc in range(KC_D):
                    nc.tensor.matmul(p_f[:, :G], w1T[:, kc, fblk * 128:(fblk + 1) * 128],
                                     lnc2T[:, kc, :], start=(kc == 0), stop=(kc == KC_D - 1))
                nc.scalar.activation(out=ffT[:, fblk, :], in_=p_f[:, :G],
                                     func=mybir.ActivationFunctionType.Relu,
                                     bias=bl1[:, fblk:fblk + 1], scale=1.0)
            return ffT

        def emit_ffn2_ln3(b, t2, lnc2, ffT):
            for nch in range(2):
                p_2 = ps.tile([128, 512], f32, tag="ps")
                nc.tensor.matmul(p_2[:G, :384], ones_mm[:, :G],
                                 bl2_row[:, nch * 384:(nch + 1) * 384],
                                 start=True, stop=False)
                for kc in range(KC_FF):
                    nc.tensor.matmul(p_2[:G, :384], ffT[:, kc, :],
                                     w2T[:, kc, nch * 384:(nch + 1) * 384],
                                     start=False, stop=(kc == KC_FF - 1))
                sl = slice(nch * 384, (nch + 1) * 384)
                nc.vector.tensor_mul(t2[:, sl], lnc2[:, sl], g2b[:, sl])
                nc.vector.tensor_add(t2[:, sl], t2[:, sl], p_2[:G, :384])
            lnc3 = lnpool.tile([G, D], f32, tag="lnbuf")
            layernorm_core(lnc3, t2, "ln3")
            return lnc3

        def emit_hT(b, lnc3):
            for kc in range(KC_D):
                transpose_cols(hT[:, kc, b, :], lnc3, kc)

        # interleaved emission: stage-2 of batch b-1 woven between
        # stage-1 pieces of batch b so LN/softmax chains hide under PE work
        xs_next = emit_x(0)
        for b in range(BL):
            xs = xs_next
            if b + 1 < BL:
                xs_next = emit_x(b + 1)
            prev = state.pop(b - 1, None)
            if prev is not None:
                t2p, lnc2p, lnc2Tp = emit_oproj_ln2(b - 1, prev)
            memT = emit_embed(b, xs)
            if prev is not None:
                ffTp = emit_ffn1(b - 1, lnc2Tp)
            kT = emit_kT(b, memT)
            if prev is not None:
                lnc3p = emit_ffn2_ln3(b - 1, t2p, lnc2p, ffTp)
            v_sb = emit_v(b, memT)
            if prev is not None:
                emit_hT(b - 1, lnc3p)
            state[b] = emit_attn(b, kT, v_sb)

        bl = BL - 1
        ctxT_l = state.pop(bl)
        t2l, lnc2l, lnc2Tl = emit_oproj_ln2(bl, ctxT_l)
        ffTl = emit_ffn1(bl, lnc2Tl)
        lnc3l = emit_ffn2_ln3(bl, t2l, lnc2l, ffTl)
        emit_hT(bl, lnc3l)

        # ---------- grouped FC ----------
        dup_view = dram("dup").rearrange("(kc p) g f -> p kc g f", p=128)
        out_flat = dram("out")
        g0 = 0
        while g0 < G:
            ng = min(NGRP, G - g0)
            dup_sb = dupp.tile([128, KC_D, NGRP, DF], bf16, tag="dup")
            gsl = slice(g0, g0 + ng)
            nc.scalar.dma_start(out=dup_sb[:, :KC_D // 2, :ng, :],
                                in_=dup_view[:, :KC_D // 2, gsl, :])
            nc.sync.dma_start(out=dup_sb[:, KC_D // 2:, :ng, :],
                              in_=dup_view[:, KC_D // 2:, gsl, :])
            db2_sb = dupp.tile([1, NGRP, DF], bf16, tag="db2c")
            nc.sync.dma_start(out=db2_sb[:, :ng, :], in_=dram("db2")[gsl, :])
            lchunk = work.tile([BL, NGRP, DF], f32, tag="lchunk")
            for gi in range(ng):
                g = g0 + gi
                p_g = ps.tile([128, 512], f32, tag="ps")
                nc.tensor.matmul(p_g[:BL, :DF], ones_bf[:, :BL],
                                 db2_sb[:, gi, :], start=True, stop=False)
                for kc in range(KC_D):
                    nc.tensor.matmul(p_g[:BL, :DF], hT[:, kc, :, g],
                                     dup_sb[:, kc, gi, :],
                                     start=False, stop=(kc == KC_D - 1))
                nc.vector.tensor_copy(out=lchunk[:, gi, :], in_=p_g[:BL, :DF])
            c0 = g0 * DF
            wout = min(ng * DF, NCLS - c0)
            nc.gpsimd.dma_start(
                out=out_flat[:, c0:c0 + wout],
                in_=lchunk.rearrange("b gi f -> b (gi f)")[:, :wout])
            g0 += ng


# ---------------- host side ----------------

_CACHED = {}


def _prep_inputs(inputs):
    f = np.float64
    w_embed = inputs["w_embed"].astype(f)
    wq, wk, wv, wo = (inputs[k].astype(f) for k in ("wq", "wk", "wv", "wo"))
    bq, bk, bv, bo = (inputs[k].astype(f) for k in ("bq", "bk", "bv", "bo"))
    g1, be1 = inputs["g1"].astype(f), inputs["be1"].astype(f)
    g2, be2 = inputs["g2"].astype(f), inputs["be2"].astype(f)
    g3, be3 = inputs["g3"].astype(f), inputs["be3"].astype(f)
    w1, bl1 = inputs["w1"].astype(f), inputs["bl1"].astype(f)
    w2, bl2 = inputs["w2"].astype(f), inputs["bl2"].astype(f)
    dup_pool = inputs["dup_pool"].astype(f)
    dup_bias = inputs["dup_bias"].astype(f)

    sc = 1.0 / np.sqrt(HD)
    bo_eff = bo + wo @ bv
    bl1_eff = bl1 + w1 @ be2
    bl2_eff = bl2 + be2
    dup2 = dup_pool.transpose(1, 0, 2) * g3[:, None, None]  # [D, G, DF]
    db2 = np.concatenate([dup_bias, np.zeros(G * DF - NCLS)])  # [G*DF]
    db2 = db2 + np.einsum("d,gdf->gf", be3, dup_pool).reshape(-1)
    w1T_eff = (w1 * g2[None, :]).T  # [D, FF]

    x = np.ascontiguousarray(inputs["x"].reshape(B, C_IN, S))
    if SP > S:
        x = np.concatenate([x, np.zeros((B, C_IN, SP - S), np.float32)], axis=2)

    np32 = np.float32
    base = {
        "wembT": np.ascontiguousarray(w_embed.T).astype(NP_DT),
        "wkT": np.ascontiguousarray(wk.T).astype(NP_DT),
        "wvT": np.ascontiguousarray(wv.T).astype(NP_DT),
        "wqT": np.ascontiguousarray(wq.T * sc).astype(NP_DT),
        "woT": np.ascontiguousarray(wo.T).astype(NP_DT),
        "w1T": np.ascontiguousarray(w1T_eff).astype(NP_DT),
        "w2T": np.ascontiguousarray(w2.T).astype(NP_DT),
        "dup": np.ascontiguousarray(dup2).astype(ml_dtypes.bfloat16),
        "db2": db2.reshape(G, DF).astype(ml_dtypes.bfloat16),
        "qe": inputs["query_embed"].astype(np32),
        "g1": g1.astype(np32), "be1": be1.astype(np32), "g2": g2.astype(np32),
        "bemb": inputs["b_embed"].astype(np32),
        "bk": bk.astype(np32),
        "bq": (bq * sc).astype(np32),
        "bl1": bl1_eff.astype(np32),
        "bo_row": bo_eff.astype(NP_DT).reshape(1, D),
        "bl2_row": bl2_eff.astype(NP_DT).reshape(1, D),
        "ones_mm": np.ones((1, 128), NP_DT),
        "ones_bf": np.ones((1, 128), ml_dtypes.bfloat16),
    }
    in_maps = []
    for c in range(N_CORES):
        m = dict(base)
        m["x"] = np.ascontiguousarray(x[c * BL:(c + 1) * BL]).astype(NP_DT)
        in_maps.append(m)
    return in_maps


def get_nc():
    if "nc" not in _CACHED:
        _CACHED["nc"] = build_kernel()
    return _CACHED["nc"]


def kernel(**inputs) -> np.ndarray:
    nc = get_nc()
    in_maps = _prep_inputs(inputs)
    res = run_bass_kernel_spmd(nc, in_maps, core_ids=list(range(N_CORES)))
    return np.concatenate([res.results[c]["out"] for c in range(N_CORES)], axis=0)
